# revision 12
# baseline (speedup 1.0000x reference)
"""Trainium2 Bass kernel for CustomBSplineLayer.

Computes out[b,o] = sum_{i,g} spline(x)[b,i,g] * coef[o,i,g] where
spline is an order-3 (cubic) B-spline basis on uniform knots applied to
tanh(x).

Math used here (validated against the reference recursion):
  u = 3.5*tanh(x) + 3.5           in (0, 7)
  basis_g(u) = M4(u - g)          cardinal cubic B-spline, g = 0..7
  M4(s) = (relu(2-|s-2|)^3 - 4*relu(1-|s-2|)^3) / 6
Plane g=7 is identically zero because its support starts at u=7 ==
tanh upper bound, so only 7 of 8 planes contribute (K = 7*1024 = 7168).

Per-core layout (data-parallel over batch, 8 cores x 512 rows):
  - host pre-transposes x so tiles arrive as [i partitions, b cols]
  - the whole basis pipeline runs in bf16: ACT does tanh + the 7
    per-plane |3.5t + (1.5-g)| (Abs with scale/bias) + the two wide
    relu-affines; DVE does squares, cubes and the final subtract as
    packed-bf16 tensor_tensor ops (eligible for the 2x/4x DVE modes).
  - matmul runs in bf16 (1 col/cycle on the PE, same as fp32r) with
    coef pre-converted to bf16 on the host, which also halves the
    coef HBM traffic (the largest DMA stream) vs fp32/tf32.
  - out accumulates in PSUM fp32 across all 56 k-tiles.
"""

import sys

sys.path.insert(0, "/opt/trn_rl_repo")

import numpy as np
from contextlib import ExitStack

import concourse.bass as bass
import concourse.tile as tile
from concourse import bacc, mybir
from concourse.bass_utils import run_bass_kernel_spmd

F32 = mybir.dt.float32
BF16 = mybir.dt.bfloat16
I32 = mybir.dt.int32
AF = mybir.ActivationFunctionType
OP = mybir.AluOpType

B, I, O = 4096, 1024, 1024
G = 7                    # active basis planes (plane 7 == 0)
NCORES = 8
BC = B // NCORES         # 512 batch rows per core
IT = I // 128            # 8 i-tiles
KT = IT * G              # 56 k-tiles of 128
WID = G * BC             # 3584: wide free-dim (7 planes x 512 b)

C6 = float(6.0 ** (-1.0 / 3.0))          # folds the 1/6 into p
C46 = float((4.0 / 6.0) ** (1.0 / 3.0))  # folds the 4/6 into q
KQ = float(C46 / C6)                     # q = relu(KQ*p - C46)

MM_DT = BF16

LAST_RESULT = None  # BassKernelResults of the most recent run (for test.py)

_cache = {}


def _to_bf16(a: np.ndarray) -> np.ndarray:
    """Round fp32 -> bf16 (round-to-nearest-even), as ml_dtypes array."""
    dt = mybir.dt.np(BF16)
    return np.ascontiguousarray(a, dtype=np.float32).astype(dt)


def _build_nc(repeats: int = 1):
    nc = bacc.Bacc("TRN2", target_bir_lowering=False, debug=False)
    xT = nc.dram_tensor("xT", [I, BC], F32, kind="ExternalInput").ap()
    coefT = nc.dram_tensor("coefT", [G, I, O], MM_DT, kind="ExternalInput").ap()
    y = nc.dram_tensor("y", [BC, O], F32, kind="ExternalOutput").ap()

    with tile.TileContext(nc) as tc, ExitStack() as ctx:
        xt_pool = ctx.enter_context(tc.tile_pool(name="xt", bufs=2))
        small = ctx.enter_context(tc.tile_pool(name="small", bufs=2))
        wide = ctx.enter_context(tc.tile_pool(name="wide", bufs=2))
        spl_pool = ctx.enter_context(tc.tile_pool(name="spl", bufs=2))
        rhs_pool = ctx.enter_context(tc.tile_pool(name="rhs", bufs=1))
        out_pool = ctx.enter_context(tc.tile_pool(name="ot", bufs=2))
        psum_pool = ctx.enter_context(
            tc.tile_pool(name="psum", bufs=1, space=bass.MemorySpace.PSUM)
        )

        consts = ctx.enter_context(tc.tile_pool(name="consts", bufs=1))
        bias_a = []
        for g in range(G):
            bt = consts.tile([128, 1], F32, tag=f"bias_a{g}", name=f"bias_a{g}")
            nc.gpsimd.memset(bt[:], float(1.5 - g))
            bias_a.append(bt)
        bias_p = consts.tile([128, 1], F32, tag="bias_p", name="bias_p")
        nc.gpsimd.memset(bias_p[:], 2.0 * C6)
        bias_q = consts.tile([128, 1], F32, tag="bias_q", name="bias_q")
        nc.gpsimd.memset(bias_q[:], -C46)

        # Weights-stationary: the full bf16 coef (7*8 tiles of [128, O],
        # 112 KiB/partition) stays resident in SBUF across repeats, so the
        # steady state has no coef HBM traffic and the PE never waits on
        # rhs DMA.
        rhs_res = [
            [
                rhs_pool.tile([128, O], MM_DT, tag=f"rhs{it}_{g}", name=f"rhs{it}_{g}")
                for g in range(G)
            ]
            for it in range(IT)
        ]
        for it in range(IT):
            for g in range(G):
                nc.sync.dma_start(
                    rhs_res[it][g][:], coefT[g, it * 128 : (it + 1) * 128, :]
                )

        # 8 PSUM banks: [m-tile 0..3] x [o-half 0..1], each [128, 512] f32
        psum = [
            [
                psum_pool.tile(
                    [128, 512], F32, tag=f"ps{m}_{h}", name=f"ps{m}_{h}"
                )
                for h in range(2)
            ]
            for m in range(4)
        ]

        def emit_front(rep, it):
            """DMA + tanh + per-plane |3.5t + (1.5-g)| (all ACT)."""
            xt = xt_pool.tile([128, BC], F32, tag="xt", name=f"xt{rep}_{it}")
            nc.sync.dma_start(xt[:], xT[it * 128 : (it + 1) * 128, :])
            t = small.tile([128, BC], BF16, tag="t", name=f"t{rep}_{it}")
            nc.scalar.activation(t[:], xt[:], AF.Tanh)
            aw = wide.tile([128, WID], BF16, tag="a", name=f"aw{rep}_{it}")
            for g in range(G):
                nc.scalar.activation(
                    aw[:, g * BC : (g + 1) * BC],
                    t[:],
                    AF.Abs,
                    bias=bias_a[g][:],
                    scale=3.5,
                )
            return aw

        def emit_mids(rep, it, aw, chunks=1):
            """ACT relu-affines: p = relu(2C6 - C6*a), q = relu(KQ*p - C46)."""
            pw = wide.tile([128, WID], BF16, tag="p", name=f"pw{rep}_{it}")
            qw = wide.tile([128, WID], BF16, tag="q", name=f"qw{rep}_{it}")
            cw = WID // chunks
            for c in range(chunks):
                s = slice(c * cw, (c + 1) * cw)
                nc.scalar.activation(
                    pw[:, s], aw[:, s], AF.Relu, bias=bias_p[:], scale=-C6
                )
                nc.scalar.activation(
                    qw[:, s], pw[:, s], AF.Relu, bias=bias_q[:], scale=KQ
                )
            return pw, qw, aw

        def emit_cubes(rep, it, mids, chunks=1):
            """DVE bf16 stage: squares, cubes (in place), spl = p3 - q3."""
            pw, qw, aw = mids
            p2 = wide.tile([128, WID], BF16, tag="p2", name=f"p2{rep}_{it}")
            q2 = aw  # aw is dead after the relus; reuse its buffer (SBUF)
            spl = spl_pool.tile([128, WID], MM_DT, tag="spl", name=f"spl{rep}_{it}")
            cw = WID // chunks
            for c in range(chunks):
                s = slice(c * cw, (c + 1) * cw)
                nc.vector.tensor_tensor(p2[:, s], pw[:, s], pw[:, s], OP.mult)
                nc.vector.tensor_tensor(q2[:, s], qw[:, s], qw[:, s], OP.mult)
                nc.vector.tensor_tensor(p2[:, s], p2[:, s], pw[:, s], OP.mult)
                nc.vector.tensor_tensor(q2[:, s], q2[:, s], qw[:, s], OP.mult)
                nc.vector.tensor_tensor(spl[:, s], p2[:, s], q2[:, s], OP.subtract)
            return spl

        def emit_matmuls(rep, it, spl, kt):
            for g in range(G):
                rhs = rhs_res[it][g]
                first = kt == 0
                last = kt == KT - 1
                for m in range(4):
                    lhsT = spl[:, g * BC + m * 128 : g * BC + (m + 1) * 128]
                    for h in range(2):
                        nc.tensor.matmul(
                            psum[m][h][:],
                            lhsT,
                            rhs[:, h * 512 : (h + 1) * 512],
                            start=first,
                            stop=last,
                        )
                kt += 1
            return kt

        for _rep in range(repeats):
            # software-pipelined emission: ACT runs front+mids of tile it
            # while the DVE finishes the cubes of it-1, whose matmuls
            # follow immediately.
            kt = 0
            # i-tile 0 runs per-plane (chunks=G) so its first matmuls can
            # start earlier (deps are tracked per slice); later tiles use
            # full-wide ops.
            ch0 = G if _rep == 0 else 1
            aw = emit_front(_rep, 0)
            mids = emit_mids(_rep, 0, aw, chunks=ch0)
            for it in range(1, IT):
                aw = emit_front(_rep, it)
                prev_mids = mids
                mids = emit_mids(_rep, it, aw)
                spl = emit_cubes(_rep, it - 1, prev_mids, chunks=ch0 if it == 1 else 1)
                kt = emit_matmuls(_rep, it - 1, spl, kt)
            spl = emit_cubes(_rep, IT - 1, mids)
            kt = emit_matmuls(_rep, IT - 1, spl, kt)

            for m in range(4):
                ot = out_pool.tile([128, O], F32, tag="ot", name=f"ot{_rep}_{m}")
                for h in range(2):
                    nc.scalar.copy(ot[:, h * 512 : (h + 1) * 512], psum[m][h][:])
                nc.sync.dma_start(y[m * 128 : (m + 1) * 128, :], ot[:])

    nc.compile()
    return nc


def prep_in_maps(x: np.ndarray, coef: np.ndarray):
    """Host-side prep shared by kernel() and test.py."""
    xT = np.ascontiguousarray(np.asarray(x, dtype=np.float32).T)  # [I, B]
    coefT = _to_bf16(
        np.ascontiguousarray(
            np.asarray(coef, dtype=np.float32).transpose(2, 1, 0)[:G]
        )
    )  # [7, I, O] bf16
    return [
        {
            "xT": np.ascontiguousarray(xT[:, c * BC : (c + 1) * BC]),
            "coefT": coefT,
        }
        for c in range(NCORES)
    ]


def kernel(x: np.ndarray, coef: np.ndarray) -> np.ndarray:
    global LAST_RESULT
    x = np.asarray(x, dtype=np.float32)
    coef = np.asarray(coef, dtype=np.float32)
    assert x.shape == (B, I) and coef.shape == (O, I, 8)

    if "nc" not in _cache:
        _cache["nc"] = _build_nc()
    nc = _cache["nc"]

    in_maps = prep_in_maps(x, coef)
    res = run_bass_kernel_spmd(nc, in_maps, list(range(NCORES)))
    LAST_RESULT = res
    out = np.concatenate([res.results[c]["y"] for c in range(NCORES)], axis=0)
    return np.ascontiguousarray(out.astype(np.float32))


if __name__ == "__main__":
    rng = np.random.default_rng(0)
    x = rng.standard_normal((B, I), dtype=np.float32)
    coef = rng.standard_normal((O, I, 8), dtype=np.float32) * 0.1
    out = kernel(x, coef)
    print("out", out.shape, out.dtype, float(np.abs(out).max()))


# revision 16
# speedup vs baseline: 1.0303x; 1.0303x over previous
"""Trainium2 Bass kernel for CustomBSplineLayer.

Computes out[b,o] = sum_{i,g} spline(x)[b,i,g] * coef[o,i,g] where
spline is an order-3 (cubic) B-spline basis on uniform knots applied to
tanh(x).

Math used here (validated against the reference recursion):
  u = 3.5*tanh(x) + 3.5           in (0, 7)
  basis_g(u) = M4(u - g)          cardinal cubic B-spline, g = 0..7
  M4(s) = (relu(2-|s-2|)^3 - 4*relu(1-|s-2|)^3) / 6
Plane g=7 is identically zero because its support starts at u=7 ==
tanh upper bound, so only 7 of 8 planes contribute (K = 7*1024 = 7168).

Per-core layout (data-parallel over batch, 8 cores x 512 rows):
  - host pre-transposes x so tiles arrive as [i partitions, b cols]
  - the whole basis pipeline runs in bf16: ACT does tanh + the 7
    per-plane |3.5t + (1.5-g)| (Abs with scale/bias) + the two wide
    relu-affines; DVE does squares, cubes and the final subtract as
    packed-bf16 tensor_tensor ops (eligible for the 2x/4x DVE modes).
  - matmul runs in bf16 (1 col/cycle on the PE, same as fp32r) with
    coef pre-converted to bf16 on the host, which also halves the
    coef HBM traffic (the largest DMA stream) vs fp32/tf32.
  - out accumulates in PSUM fp32 across all 56 k-tiles.
"""

import sys

sys.path.insert(0, "/opt/trn_rl_repo")

import numpy as np
from contextlib import ExitStack

import concourse.bass as bass
import concourse.tile as tile
from concourse import bacc, mybir
from concourse.bass_utils import run_bass_kernel_spmd

F32 = mybir.dt.float32
BF16 = mybir.dt.bfloat16
I32 = mybir.dt.int32
AF = mybir.ActivationFunctionType
OP = mybir.AluOpType

B, I, O = 4096, 1024, 1024
G = 7                    # active basis planes (plane 7 == 0)
NCORES = 8
BC = B // NCORES         # 512 batch rows per core
IT = I // 128            # 8 i-tiles
KT = IT * G              # 56 k-tiles of 128
WID = G * BC             # 3584: wide free-dim (7 planes x 512 b)

C6 = float(6.0 ** (-1.0 / 3.0))          # folds the 1/6 into p
C46 = float((4.0 / 6.0) ** (1.0 / 3.0))  # folds the 4/6 into q
KQ = float(C46 / C6)                     # q = relu(KQ*p - C46)

MM_DT = BF16

LAST_RESULT = None  # BassKernelResults of the most recent run (for test.py)

_cache = {}


def _to_bf16(a: np.ndarray) -> np.ndarray:
    """Round fp32 -> bf16 (round-to-nearest-even), as ml_dtypes array."""
    dt = mybir.dt.np(BF16)
    return np.ascontiguousarray(a, dtype=np.float32).astype(dt)


def _build_nc(repeats: int = 1):
    nc = bacc.Bacc("TRN2", target_bir_lowering=False, debug=False)
    xT = nc.dram_tensor("xT", [I, BC], F32, kind="ExternalInput").ap()
    coefT = nc.dram_tensor("coefT", [G, I, O], MM_DT, kind="ExternalInput").ap()
    y = nc.dram_tensor("y", [BC, O], F32, kind="ExternalOutput").ap()

    with tile.TileContext(nc) as tc, ExitStack() as ctx:
        xt_pool = ctx.enter_context(tc.tile_pool(name="xt", bufs=2))
        small = ctx.enter_context(tc.tile_pool(name="small", bufs=2))
        wide = ctx.enter_context(tc.tile_pool(name="wide", bufs=2))
        spl_pool = ctx.enter_context(tc.tile_pool(name="spl", bufs=2))
        rhs_pool = ctx.enter_context(tc.tile_pool(name="rhs", bufs=6))
        out_pool = ctx.enter_context(tc.tile_pool(name="ot", bufs=2))
        psum_pool = ctx.enter_context(
            tc.tile_pool(name="psum", bufs=1, space=bass.MemorySpace.PSUM)
        )

        consts = ctx.enter_context(tc.tile_pool(name="consts", bufs=1))
        bias_a = []
        for g in range(G):
            bt = consts.tile([128, 1], F32, tag=f"bias_a{g}", name=f"bias_a{g}")
            nc.gpsimd.memset(bt[:], float(1.5 - g))
            bias_a.append(bt)
        bias_p = consts.tile([128, 1], F32, tag="bias_p", name="bias_p")
        nc.gpsimd.memset(bias_p[:], 2.0 * C6)
        bias_q = consts.tile([128, 1], F32, tag="bias_q", name="bias_q")
        nc.gpsimd.memset(bias_q[:], -C46)

        # 8 PSUM banks: [m-tile 0..3] x [o-half 0..1], each [128, 512] f32
        psum = [
            [
                psum_pool.tile(
                    [128, 512], F32, tag=f"ps{m}_{h}", name=f"ps{m}_{h}"
                )
                for h in range(2)
            ]
            for m in range(4)
        ]

        def emit_front(rep, it):
            """DMA + tanh + per-plane |3.5t + (1.5-g)| (all ACT)."""
            xt = xt_pool.tile([128, BC], F32, tag="xt", name=f"xt{rep}_{it}")
            nc.sync.dma_start(xt[:], xT[it * 128 : (it + 1) * 128, :])
            t = small.tile([128, BC], BF16, tag="t", name=f"t{rep}_{it}")
            nc.scalar.activation(t[:], xt[:], AF.Tanh)
            aw = wide.tile([128, WID], BF16, tag="a", name=f"aw{rep}_{it}")
            for g in range(G):
                nc.scalar.activation(
                    aw[:, g * BC : (g + 1) * BC],
                    t[:],
                    AF.Abs,
                    bias=bias_a[g][:],
                    scale=3.5,
                )
            return aw

        def emit_mids(rep, it, aw, chunks=1):
            """ACT relu-affines: p = relu(2C6 - C6*a), q = relu(KQ*p - C46)."""
            pw = wide.tile([128, WID], BF16, tag="p", name=f"pw{rep}_{it}")
            qw = wide.tile([128, WID], BF16, tag="q", name=f"qw{rep}_{it}")
            cw = WID // chunks
            for c in range(chunks):
                s = slice(c * cw, (c + 1) * cw)
                nc.scalar.activation(
                    pw[:, s], aw[:, s], AF.Relu, bias=bias_p[:], scale=-C6
                )
                nc.scalar.activation(
                    qw[:, s], pw[:, s], AF.Relu, bias=bias_q[:], scale=KQ
                )
            return pw, qw, aw

        def emit_cubes(rep, it, mids, chunks=1):
            """DVE bf16 stage: squares, cubes (in place), spl = p3 - q3."""
            pw, qw, aw = mids
            p2 = wide.tile([128, WID], BF16, tag="p2", name=f"p2{rep}_{it}")
            q2 = wide.tile([128, WID], BF16, tag="q2", name=f"q2{rep}_{it}")
            spl = spl_pool.tile([128, WID], MM_DT, tag="spl", name=f"spl{rep}_{it}")
            cw = WID // chunks
            for c in range(chunks):
                s = slice(c * cw, (c + 1) * cw)
                nc.vector.tensor_tensor(p2[:, s], pw[:, s], pw[:, s], OP.mult)
                nc.vector.tensor_tensor(q2[:, s], qw[:, s], qw[:, s], OP.mult)
                nc.vector.tensor_tensor(p2[:, s], p2[:, s], pw[:, s], OP.mult)
                nc.vector.tensor_tensor(q2[:, s], q2[:, s], qw[:, s], OP.mult)
                nc.vector.tensor_tensor(spl[:, s], p2[:, s], q2[:, s], OP.subtract)
            return spl

        def emit_matmuls(rep, it, spl, kt):
            for g in range(G):
                rhs = rhs_pool.tile(
                    [128, O], MM_DT, tag="rhs", name=f"rhs{rep}_{it}_{g}"
                )
                nc.sync.dma_start(rhs[:], coefT[g, it * 128 : (it + 1) * 128, :])
                first = kt == 0
                last = kt == KT - 1
                for m in range(4):
                    lhsT = spl[:, g * BC + m * 128 : g * BC + (m + 1) * 128]
                    for h in range(2):
                        nc.tensor.matmul(
                            psum[m][h][:],
                            lhsT,
                            rhs[:, h * 512 : (h + 1) * 512],
                            start=first,
                            stop=last,
                        )
                kt += 1
            return kt

        for _rep in range(repeats):
            # software-pipelined emission: ACT runs front+mids of tile it
            # while the DVE finishes the cubes of it-1, whose matmuls
            # follow immediately.
            kt = 0
            # i-tile 0 runs per-plane (chunks=G) so its first matmuls can
            # start earlier (deps are tracked per slice); later tiles use
            # full-wide ops.
            ch0 = G if _rep == 0 else 1
            aw = emit_front(_rep, 0)
            mids = emit_mids(_rep, 0, aw, chunks=ch0)
            for it in range(1, IT):
                aw = emit_front(_rep, it)
                prev_mids = mids
                mids = emit_mids(_rep, it, aw)
                spl = emit_cubes(_rep, it - 1, prev_mids, chunks=ch0 if it == 1 else 1)
                kt = emit_matmuls(_rep, it - 1, spl, kt)
            spl = emit_cubes(_rep, IT - 1, mids)
            kt = emit_matmuls(_rep, IT - 1, spl, kt)

            for m in range(4):
                ot = out_pool.tile([128, O], F32, tag="ot", name=f"ot{_rep}_{m}")
                for h in range(2):
                    nc.scalar.copy(ot[:, h * 512 : (h + 1) * 512], psum[m][h][:])
                nc.sync.dma_start(y[m * 128 : (m + 1) * 128, :], ot[:])

    nc.compile()
    return nc


def prep_in_maps(x: np.ndarray, coef: np.ndarray):
    """Host-side prep shared by kernel() and test.py."""
    xT = np.ascontiguousarray(np.asarray(x, dtype=np.float32).T)  # [I, B]
    coefT = _to_bf16(
        np.ascontiguousarray(
            np.asarray(coef, dtype=np.float32).transpose(2, 1, 0)[:G]
        )
    )  # [7, I, O] bf16
    return [
        {
            "xT": np.ascontiguousarray(xT[:, c * BC : (c + 1) * BC]),
            "coefT": coefT,
        }
        for c in range(NCORES)
    ]


def kernel(x: np.ndarray, coef: np.ndarray) -> np.ndarray:
    global LAST_RESULT
    x = np.asarray(x, dtype=np.float32)
    coef = np.asarray(coef, dtype=np.float32)
    assert x.shape == (B, I) and coef.shape == (O, I, 8)

    if "nc" not in _cache:
        _cache["nc"] = _build_nc()
    nc = _cache["nc"]

    in_maps = prep_in_maps(x, coef)
    res = run_bass_kernel_spmd(nc, in_maps, list(range(NCORES)))
    LAST_RESULT = res
    out = np.concatenate([res.results[c]["y"] for c in range(NCORES)], axis=0)
    return np.ascontiguousarray(out.astype(np.float32))


if __name__ == "__main__":
    rng = np.random.default_rng(0)
    x = rng.standard_normal((B, I), dtype=np.float32)
    coef = rng.standard_normal((O, I, 8), dtype=np.float32) * 0.1
    out = kernel(x, coef)
    print("out", out.shape, out.dtype, float(np.abs(out).max()))


# revision 17
# speedup vs baseline: 1.4150x; 1.3733x over previous
"""Trainium2 Bass kernel for CustomBSplineLayer.

Computes out[b,o] = sum_{i,g} spline(x)[b,i,g] * coef[o,i,g] where
spline is an order-3 (cubic) B-spline basis on uniform knots applied to
tanh(x).

Math used here (validated against the reference recursion):
  u = 3.5*tanh(x) + 3.5           in (0, 7)
  basis_g(u) = M4(u - g)          cardinal cubic B-spline, g = 0..7
  M4(s) = (relu(2-|s-2|)^3 - 4*relu(1-|s-2|)^3) / 6
Plane g=7 is identically zero because its support starts at u=7 ==
tanh upper bound, so only 7 of 8 planes contribute (K = 7*1024 = 7168).

Per-core layout (data-parallel over batch, 8 cores x 512 rows):
  - host pre-transposes x so tiles arrive as [i partitions, b cols]
  - the whole basis pipeline runs in bf16: ACT does tanh + the 7
    per-plane |3.5t + (1.5-g)| (Abs with scale/bias) + the two wide
    relu-affines; DVE does squares, cubes and the final subtract as
    packed-bf16 tensor_tensor ops (eligible for the 2x/4x DVE modes).
  - matmul runs in bf16 (1 col/cycle on the PE, same as fp32r) with
    coef pre-converted to bf16 on the host, which also halves the
    coef HBM traffic (the largest DMA stream) vs fp32/tf32.
  - out accumulates in PSUM fp32 across all 56 k-tiles.
"""

import sys

sys.path.insert(0, "/opt/trn_rl_repo")

import numpy as np
from contextlib import ExitStack

import concourse.bass as bass
import concourse.tile as tile
from concourse import bacc, mybir
from concourse.bass_utils import run_bass_kernel_spmd

F32 = mybir.dt.float32
BF16 = mybir.dt.bfloat16
I32 = mybir.dt.int32
AF = mybir.ActivationFunctionType
OP = mybir.AluOpType

B, I, O = 4096, 1024, 1024
G = 7                    # active basis planes (plane 7 == 0)
NCORES = 8
BC = B // NCORES         # 512 batch rows per core
IT = I // 128            # 8 i-tiles
KT = IT * G              # 56 k-tiles of 128
WID = G * BC             # 3584: wide free-dim (7 planes x 512 b)

C6 = float(6.0 ** (-1.0 / 3.0))          # folds the 1/6 into p
C46 = float((4.0 / 6.0) ** (1.0 / 3.0))  # folds the 4/6 into q
KQ = float(C46 / C6)                     # q = relu(KQ*p - C46)

MM_DT = BF16

LAST_RESULT = None  # BassKernelResults of the most recent run (for test.py)

_cache = {}


def _to_bf16(a: np.ndarray) -> np.ndarray:
    """Round fp32 -> bf16 (round-to-nearest-even), as ml_dtypes array."""
    dt = mybir.dt.np(BF16)
    return np.ascontiguousarray(a, dtype=np.float32).astype(dt)


def _build_nc(repeats: int = 1):
    nc = bacc.Bacc("TRN2", target_bir_lowering=False, debug=False)
    xT = nc.dram_tensor("xT", [I, BC], F32, kind="ExternalInput").ap()
    coefT = nc.dram_tensor("coefT", [G, I, O], MM_DT, kind="ExternalInput").ap()
    y = nc.dram_tensor("y", [BC, O], F32, kind="ExternalOutput").ap()

    with tile.TileContext(nc) as tc, ExitStack() as ctx:
        xt_pool = ctx.enter_context(tc.tile_pool(name="xt", bufs=2))
        small = ctx.enter_context(tc.tile_pool(name="small", bufs=2))
        wide = ctx.enter_context(tc.tile_pool(name="wide", bufs=2))
        spl_pool = ctx.enter_context(tc.tile_pool(name="spl", bufs=2))
        rhs_pool = ctx.enter_context(tc.tile_pool(name="rhs", bufs=3))
        out_pool = ctx.enter_context(tc.tile_pool(name="ot", bufs=2))
        psum_pool = ctx.enter_context(
            tc.tile_pool(name="psum", bufs=1, space=bass.MemorySpace.PSUM)
        )

        consts = ctx.enter_context(tc.tile_pool(name="consts", bufs=1))
        bias_a = []
        for g in range(G):
            bt = consts.tile([128, 1], F32, tag=f"bias_a{g}", name=f"bias_a{g}")
            nc.gpsimd.memset(bt[:], float(1.5 - g))
            bias_a.append(bt)
        bias_p = consts.tile([128, 1], F32, tag="bias_p", name="bias_p")
        nc.gpsimd.memset(bias_p[:], 2.0 * C6)
        bias_q = consts.tile([128, 1], F32, tag="bias_q", name="bias_q")
        nc.gpsimd.memset(bias_q[:], -C46)

        # 8 PSUM banks: [m-tile 0..3] x [o-half 0..1], each [128, 512] f32
        psum = [
            [
                psum_pool.tile(
                    [128, 512], F32, tag=f"ps{m}_{h}", name=f"ps{m}_{h}"
                )
                for h in range(2)
            ]
            for m in range(4)
        ]

        def emit_front(rep, it):
            """DMA + tanh + per-plane |3.5t + (1.5-g)| (all ACT)."""
            xt = xt_pool.tile([128, BC], F32, tag="xt", name=f"xt{rep}_{it}")
            nc.sync.dma_start(xt[:], xT[it * 128 : (it + 1) * 128, :])
            t = small.tile([128, BC], BF16, tag="t", name=f"t{rep}_{it}")
            nc.scalar.activation(t[:], xt[:], AF.Tanh)
            aw = wide.tile([128, WID], BF16, tag="a", name=f"aw{rep}_{it}")
            for g in range(G):
                nc.scalar.activation(
                    aw[:, g * BC : (g + 1) * BC],
                    t[:],
                    AF.Abs,
                    bias=bias_a[g][:],
                    scale=3.5,
                )
            return aw

        def emit_mids(rep, it, aw, chunks=1):
            """ACT relu-affines: p = relu(2C6 - C6*a), q = relu(KQ*p - C46)."""
            pw = wide.tile([128, WID], BF16, tag="p", name=f"pw{rep}_{it}")
            qw = wide.tile([128, WID], BF16, tag="q", name=f"qw{rep}_{it}")
            cw = WID // chunks
            for c in range(chunks):
                s = slice(c * cw, (c + 1) * cw)
                nc.scalar.activation(
                    pw[:, s], aw[:, s], AF.Relu, bias=bias_p[:], scale=-C6
                )
                nc.scalar.activation(
                    qw[:, s], pw[:, s], AF.Relu, bias=bias_q[:], scale=KQ
                )
            return pw, qw, aw

        def emit_cubes(rep, it, mids, chunks=1):
            """DVE bf16 stage: squares, cubes (in place), spl = p3 - q3."""
            pw, qw, aw = mids
            p2 = wide.tile([128, WID], BF16, tag="p2", name=f"p2{rep}_{it}")
            q2 = wide.tile([128, WID], BF16, tag="q2", name=f"q2{rep}_{it}")
            spl = spl_pool.tile([128, WID], MM_DT, tag="spl", name=f"spl{rep}_{it}")
            cw = WID // chunks
            for c in range(chunks):
                s = slice(c * cw, (c + 1) * cw)
                nc.vector.tensor_tensor(p2[:, s], pw[:, s], pw[:, s], OP.mult)
                nc.vector.tensor_tensor(q2[:, s], qw[:, s], qw[:, s], OP.mult)
                nc.vector.tensor_tensor(p2[:, s], p2[:, s], pw[:, s], OP.mult)
                nc.vector.tensor_tensor(q2[:, s], q2[:, s], qw[:, s], OP.mult)
                nc.vector.tensor_tensor(spl[:, s], p2[:, s], q2[:, s], OP.subtract)
            return spl

        def emit_matmuls(rep, it, spl, kt):
            for g in range(G):
                rhs = rhs_pool.tile(
                    [128, O], MM_DT, tag="rhs", name=f"rhs{rep}_{it}_{g}"
                )
                nc.sync.dma_start(rhs[:], coefT[g, it * 128 : (it + 1) * 128, :])
                first = kt == 0
                last = kt == KT - 1
                for m in range(4):
                    lhsT = spl[:, g * BC + m * 128 : g * BC + (m + 1) * 128]
                    for h in range(2):
                        nc.tensor.matmul(
                            psum[m][h][:],
                            lhsT,
                            rhs[:, h * 512 : (h + 1) * 512],
                            start=first,
                            stop=last,
                        )
                kt += 1
            return kt

        for _rep in range(repeats):
            # software-pipelined emission: ACT runs front+mids of tile it
            # while the DVE finishes the cubes of it-1, whose matmuls
            # follow immediately.
            kt = 0
            # i-tile 0 runs per-plane (chunks=G) so its first matmuls can
            # start earlier (deps are tracked per slice); later tiles use
            # full-wide ops.
            ch0 = G if _rep == 0 else 1
            aw = emit_front(_rep, 0)
            mids = emit_mids(_rep, 0, aw, chunks=ch0)
            for it in range(1, IT):
                aw = emit_front(_rep, it)
                prev_mids = mids
                mids = emit_mids(_rep, it, aw)
                spl = emit_cubes(_rep, it - 1, prev_mids, chunks=ch0 if it == 1 else 1)
                kt = emit_matmuls(_rep, it - 1, spl, kt)
            spl = emit_cubes(_rep, IT - 1, mids)
            kt = emit_matmuls(_rep, IT - 1, spl, kt)

            for m in range(4):
                ot = out_pool.tile([128, O], F32, tag="ot", name=f"ot{_rep}_{m}")
                for h in range(2):
                    nc.scalar.copy(ot[:, h * 512 : (h + 1) * 512], psum[m][h][:])
                nc.sync.dma_start(y[m * 128 : (m + 1) * 128, :], ot[:])

    nc.compile()
    return nc


def prep_in_maps(x: np.ndarray, coef: np.ndarray):
    """Host-side prep shared by kernel() and test.py."""
    xT = np.ascontiguousarray(np.asarray(x, dtype=np.float32).T)  # [I, B]
    coefT = _to_bf16(
        np.ascontiguousarray(
            np.asarray(coef, dtype=np.float32).transpose(2, 1, 0)[:G]
        )
    )  # [7, I, O] bf16
    return [
        {
            "xT": np.ascontiguousarray(xT[:, c * BC : (c + 1) * BC]),
            "coefT": coefT,
        }
        for c in range(NCORES)
    ]


def kernel(x: np.ndarray, coef: np.ndarray) -> np.ndarray:
    global LAST_RESULT
    x = np.asarray(x, dtype=np.float32)
    coef = np.asarray(coef, dtype=np.float32)
    assert x.shape == (B, I) and coef.shape == (O, I, 8)

    if "nc" not in _cache:
        _cache["nc"] = _build_nc()
    nc = _cache["nc"]

    in_maps = prep_in_maps(x, coef)
    res = run_bass_kernel_spmd(nc, in_maps, list(range(NCORES)))
    LAST_RESULT = res
    out = np.concatenate([res.results[c]["y"] for c in range(NCORES)], axis=0)
    return np.ascontiguousarray(out.astype(np.float32))


if __name__ == "__main__":
    rng = np.random.default_rng(0)
    x = rng.standard_normal((B, I), dtype=np.float32)
    coef = rng.standard_normal((O, I, 8), dtype=np.float32) * 0.1
    out = kernel(x, coef)
    print("out", out.shape, out.dtype, float(np.abs(out).max()))
